# revision 88
# baseline (speedup 1.0000x reference)
"""Trainium2 Bass kernel for nn_Attention_85005992722686.

Batch x head-group sharded causal attention over 8 NeuronCores.
Core c owns batch c//4 and heads {4g..4g+3} (g = c%4), processed as two
head-pair passes hp in {0,1} (the same two-pass pipeline shape a
2-batch schedule would use, so PSUM stays at 8 banks).  Splitting the
batch across the core grid halves the per-core x / fp8-x / y DMA bytes
(the cost model serializes all DMA on one device, so bytes ~= wall
time) and computes the LN statistics once instead of per pass.

Both layernorms fold into the projection weights (gamma scales the
weight columns, the mean term becomes a rank-1 PSUM correction, rstd a
per-token scale); per-core partial outputs through the row-shard of Wo
are summed on the host within each batch's 4-core group.

All matmul operands are bf16 (PSUM accumulates fp32) except the token
gram for the LN variance, which runs in fp8e4m3 DoubleRow (0.5 cyc/row,
256-deep contraction; variance averages 1024 squares so the fp8 noise
is ~0.2%).  The cost model charges a matmul `out_free_size` rows at
0.4167ns/row regardless of K and M, so the structure minimizes total
streamed output columns:

  phase A (per 512-token chunk):
    gram:    64-col token-gram diag blocks; diag = sum(x^2) per token
    v-proj:  natural layout out[t, 130] = [v_h0|v_h1|mean|pad]; the 1/D
             weight column yields token means for free
    q-proj:  natural layout [t, 128] + rank-1 mean fix, rstd applied as
             a per-partition scalar at eviction, PE-transposed to [hd,t]
    k-proj:  directly transposed (weights stationary, x moving); its
             per-token rstd rides the exp's per-partition scale
    rstd:    Newton rsqrt on GPSIMD (var is ~1, three mult/add steps)
    mean row: per-column PE transposes landing on partition 0 (no DMA)
  phase B (per 512-query chunk c4):
    S^T blocks [j, i] per (jt, head) -> exp on ACT (scale = rstd_k) ->
    tri-mask (diag) on GPSIMD
    PV in natural orientation: out[i, 65] = P-block^T @ [v_h|1], PSUM-
    accumulated over jt; col 64 is the softmax denominator; PV lags its
    S by 1 jt (h0) / 2 jts (h1) to cover the exp latency
    normalize fused (one reciprocal + one broadcast multiply per chunk)
    transpose; y = sum_hp attnT_hp^T @ Wo_hp accumulated in PSUM

A dependency-paced interleaver merges both passes' phase A and B
emission (B announces chunk needs and filler budgets, A announces chunk
completion and PE-cycle estimates) so the PE stream stays dense while
ACT digests the exps; PSUM: 3 banks S ring, 2 banks attention
accumulators, 3 banks misc ring.
"""
import sys
sys.path.insert(0, '/opt/trn_rl_repo')
import numpy as np
import ml_dtypes
import concourse.bass as bass
import concourse.bacc as bacc
import concourse.tile as tile
from concourse import mybir
from concourse.bass_utils import run_bass_kernel_spmd

F32 = mybir.dt.float32
BF16 = mybir.dt.bfloat16
FP8 = mybir.dt.float8e4
DR = mybir.MatmulPerfMode.DoubleRow
AF = mybir.ActivationFunctionType
ALU = mybir.AluOpType

B, N, D = 2, 2048, 1024
H, DH = 16, 64
EPS = 1e-5
NCORES = 8
HG = 4            # heads per core (2 head-pairs x 2 heads)
KT = D // 128     # 8 k-tiles over model dim
NT = N // 128     # 16 token tiles
NCH = N // 512    # 4 chunks of 512 tokens

STAGE = 6         # debug: 2 gram/v/stats, 3 full phase A, 4 +S/exp, 5 +PV, 6 full
FILL_CYC = 1024   # PE-cycles of deferred filler inserted after each S pair
TRACE = False
TRACE_KWARGS = {}
LAST_RESULTS = None
NPBF = ml_dtypes.bfloat16


def _build_program(with_bias):
    nc = bacc.Bacc("TRN2", target_bir_lowering=False, debug=False,
                   num_devices=NCORES)
    # ---------------- dram io (one batch, 4 heads per core) ----------------
    xt_d = nc.dram_tensor("xt", [D, N], BF16, kind="ExternalInput")
    # fp8 copy of x^T, k-tiles side by side: [p, kt*N + t] = x[kt*128+p, t]
    xf8_d = nc.dram_tensor("xf8", [128, KT * N], FP8, kind="ExternalInput")
    # host-packed per k-tile: [q_hp0 128 | k_hp0 128 | q_hp1 128 | k_hp1 128]
    wqk_d = nc.dram_tensor("wqk", [128, KT * 512], BF16, kind="ExternalInput")
    # per k-tile: [v_hp0 130 | v_hp1 130]
    wv_d = nc.dram_tensor("wv", [128, KT * 260], BF16, kind="ExternalInput")
    # [wo_hp0 D | wo_hp1 D]
    wo_d = nc.dram_tensor("wo", [128, 2 * D], BF16, kind="ExternalInput")
    # per hp at hp*640: [ncs_q 0:128 | ncs_k 128:256 | ncs_v 256:386 | ones 512:640]
    aux_d = nc.dram_tensor("aux", [1, 1280], BF16, kind="ExternalInput")
    tri_d = nc.dram_tensor("tri", [128, 128], BF16, kind="ExternalInput")
    identb_d = nc.dram_tensor("identb", [128, 128], BF16, kind="ExternalInput")
    identf_d = nc.dram_tensor("identf", [128, 128], F32, kind="ExternalInput")
    if with_bias:
        # per hp at hp*386: [bq 0:128 | bk 128:256 | bv 256:321]
        biasr_d = nc.dram_tensor("biasr", [1, 772], BF16, kind="ExternalInput")
    y_d = nc.dram_tensor("y", [N, D], BF16, kind="ExternalOutput")

    with tile.TileContext(nc) as tc:
        with tc.tile_pool(name="wpool", bufs=1) as wpool, \
             tc.tile_pool(name="xpool", bufs=2) as xpool, \
             tc.tile_pool(name="big", bufs=2) as bigp, \
             tc.tile_pool(name="small", bufs=1) as smallp, \
             tc.tile_pool(name="ppool", bufs=14) as ppool, \
             tc.tile_pool(name="psS", bufs=3, space="PSUM") as psS, \
             tc.tile_pool(name="psA", bufs=1, space="PSUM") as psA, \
             tc.tile_pool(name="psM", bufs=3, space="PSUM") as psM:

            # ---- input DMAs: half tiles pace chunk 0; xf8 (only needed
            # from chunk 1's gram on) slots in after the first half.
            xt_sb = {}
            xf8_sb = [None]

            def load_x():
                # three waves: chunk-0 quarters (all chunk 0 needs), then
                # chunk-1 quarters, the fp8 copy, then the back half --
                # chunk 0's whole chain starts ~3us earlier this way
                for wave, lo, hi in ((0, 0, 512), (1, 512, 1024),
                                     (3, 1024, 2048)):
                    for kt in range(KT):
                        t = xpool.tile([128, hi - lo], BF16,
                                       name=f"x_{kt}_{wave}",
                                       tag=f"bx{kt}w{wave}", bufs=1)
                        nc.sync.dma_start(
                            t[:], xt_d.ap()[kt * 128:(kt + 1) * 128, lo:hi])
                        xt_sb[kt, wave] = t
                    if wave == 1:
                        t8 = xpool.tile([128, KT * N], FP8, name="xf8",
                                        tag="xf8", bufs=1)
                        xf8_sb[0] = t8.rearrange("p (k t) -> p k t", t=N)
                        for s in range(4):
                            w = KT * N // 4
                            nc.sync.dma_start(
                                t8[:, s * w:(s + 1) * w],
                                xf8_d.ap()[:, s * w:(s + 1) * w])

            identf_sb = wpool.tile([128, 128], F32, name="identf_sb")
            nc.scalar.dma_start(identf_sb[:], identf_d.ap()[:, :])
            wv_sb = wpool.tile([128, KT * 260], BF16, name="wv_sb")
            nc.scalar.dma_start(wv_sb[:], wv_d.ap()[:, :])
            load_x()
            wqk_sb = wpool.tile([128, KT * 512], BF16, name="wqk_sb")
            nc.scalar.dma_start(wqk_sb[:], wqk_d.ap()[:, :])
            aux_sb = wpool.tile([1, 1280], BF16, name="aux_sb")
            nc.scalar.dma_start(aux_sb[:], aux_d.ap()[:, :])
            identb_sb = wpool.tile([128, 128], BF16, name="identb_sb")
            nc.scalar.dma_start(identb_sb[:], identb_d.ap()[:, :])
            # tri (first diag exp ~17us) and wo (first outproj ~25us) ride
            # the sync queue BEHIND the x stream so their wire time doesn't
            # compete with the startup waves on the serialized DMA device
            tri_sb = wpool.tile([128, 128], BF16, name="tri_sb")
            nc.sync.dma_start(tri_sb[:], tri_d.ap()[:, :])
            wo_sb = wpool.tile([128, 2 * D], BF16, name="wo_sb")
            nc.sync.dma_start(wo_sb[:], wo_d.ap()[:, :])
            if with_bias:
                bias_sb = wpool.tile([1, 772], BF16, name="bias_sb")
                nc.scalar.dma_start(bias_sb[:], biasr_d.ap()[:, :])

            def xtv(kt, lo, hi):
                if lo < 512:
                    return xt_sb[kt, 0][:, lo:hi]
                if lo < 1024:
                    return xt_sb[kt, 1][:, lo - 512:hi - 512]
                return xt_sb[kt, 3][:, lo - 1024:hi - 1024]

            def wqkv(kt, hp, which):
                o = kt * 512 + hp * 256 + which * 128
                return wqk_sb[:, o:o + 128]

            def wvv(kt, hp):
                o = kt * 260 + hp * 130
                return wv_sb[:, o:o + 130]

            def auxv(hp, which):   # 0 q, 1 k, 2 v(130)
                o = hp * 640 + which * 128
                return aux_sb[0:1, o:o + (130 if which == 2 else 128)]

            def biasv(hp, which):
                o = hp * 386 + which * 128
                return bias_sb[0:1, o:o + (130 if which == 2 else 128)]

            # ---- per head-pair state ----
            qT = {}; kTt = {}; v_nat = {}; attnT = {}
            stats = {}; mrow = {}; drow = {}
            for hp in range(2):
                qT[hp] = bigp.tile([128, N], BF16, name=f"qT{hp}", tag="qT")
                kTt[hp] = bigp.tile([128, N], BF16, name=f"kT{hp}", tag="kT")
                v_nat[hp] = bigp.tile([128, NT * 130], BF16, name=f"vn{hp}",
                                      tag="vn")
                attnT[hp] = bigp.tile([128, N], BF16, name=f"aT{hp}",
                                      tag="aT")
                # ones cols for the PV denominators
                vv = v_nat[hp].rearrange("p (n c) -> p n c", c=65)
                nc.vector.memset(vv[:, :, 64:65], 1.0)

            # ========= phase A (projections + shared LN stats) =========
            def emit_gram(c):
                """token-gram diag blocks in fp8 DoubleRow: 0.5 cyc/row and
                256-deep contraction per matmul (4 pair-matmuls cover D)."""
                g_ps = psM.tile([128, 512], F32, name=f"g_{c}", tag="m")
                xf = xf8_sb[0]
                # pair-major: DMA split s carries exactly k-tile pair s, so
                # the first accumulation pass starts on the first arrival
                for pr in range(KT // 2):
                    for i in range(4):
                        t0 = c * 512 + i * 128
                        for g in range(2):
                            nc.tensor.matmul(
                                g_ps[:, (i * 2 + g) * 64:(i * 2 + g + 1) * 64],
                                xf[:, 2 * pr:2 * pr + 2, t0:t0 + 128],
                                xf[:, 2 * pr:2 * pr + 2,
                                   t0 + g * 64:t0 + g * 64 + 64],
                                start=(pr == 0 and i == 0 and g == 0),
                                stop=(pr == KT // 2 - 1 and i == 3 and g == 1),
                                perf_mode=DR,
                                skip_group_check=True)
                return g_ps

            def emit_vproj(hp, c, half):
                """2 token tiles (half=0: tiles 0,1; half=1: tiles 2,3);
                per-tile cols: [v_h0 64 | v_h1 64 | mean | pad] = 130"""
                v_ps = psM.tile([128, 260], F32, name=f"v{hp}_{c}_{half}",
                                tag="m")
                for li in range(2):
                    i = half * 2 + li
                    t0 = c * 512 + i * 128
                    for kt in range(KT):
                        nc.tensor.matmul(
                            v_ps[:, li * 130:li * 130 + 130],
                            xtv(kt, t0, t0 + 128),
                            wvv(kt, hp),
                            start=(li == 0 and kt == 0), stop=False,
                            skip_group_check=True)
                return v_ps

            def emit_diag(c, g_ps):
                # stats cols: 0:4 mean, 4:8 rstd, 8:12 var, 12:16 std
                st = smallp.tile([128, 16], F32, name=f"st_{c}",
                                 tag="stats", bufs=4)
                stats[c] = st
                scr = smallp.tile([64, 64], F32, name=f"scr_{c}",
                                  tag="scr", bufs=2)
                for i in range(4):
                    for g in range(2):
                        nc.vector.scalar_tensor_tensor(
                            out=scr[:],
                            in0=g_ps[g * 64:(g + 1) * 64,
                                     (i * 2 + g) * 64:(i * 2 + g + 1) * 64],
                            scalar=1.0 / D,
                            in1=identf_sb[0:64, 0:64],
                            op0=ALU.mult, op1=ALU.mult,
                            accum_out=st[g * 64:(g + 1) * 64, 8 + i:9 + i])

            def emit_meanvar(c, v_a, v_b):
                st = stats[c]
                for half, v_ps in ((0, v_a), (1, v_b)):
                    vv = v_ps.rearrange("p (n c) -> p n c", c=130)
                    nc.vector.tensor_copy(
                        st[:, 2 * half:2 * half + 2]
                        .rearrange("p (n c) -> p n c", c=1),
                        vv[:, :, 128:129])
                sq = smallp.tile([128, 4], F32, name=f"sq_{c}", tag="sq",
                                 bufs=2)
                nc.vector.tensor_mul(sq[:], st[:, 0:4], st[:, 0:4])
                nc.vector.scalar_tensor_tensor(
                    out=st[:, 8:12], in0=st[:, 8:12], scalar=EPS, in1=sq[:],
                    op0=ALU.add, op1=ALU.subtract)
                # rstd = rsqrt(var) by Newton iteration on GPSIMD (mult/add
                # only).  LN input is unit-normal so var+eps is within
                # [0.7, 1.4]; three steps from y0=1 give ~1e-7 accuracy and
                # keep both ACT (exp-bound) and DVE off this chain.
                y = st[:, 4:8]
                t = smallp.tile([128, 4], F32, name=f"nw_{c}", tag="nw",
                                bufs=2)
                nc.gpsimd.tensor_scalar(out=y, in0=st[:, 8:12],
                                        scalar1=-0.5, scalar2=1.5,
                                        op0=ALU.mult, op1=ALU.add)
                for _ in range(2):
                    nc.gpsimd.tensor_mul(t[:], y, y)
                    nc.gpsimd.tensor_mul(t[:], t[:], st[:, 8:12])
                    nc.gpsimd.tensor_scalar(out=t[:], in0=t[:],
                                            scalar1=-0.5, scalar2=1.5,
                                            op0=ALU.mult, op1=ALU.add)
                    nc.gpsimd.tensor_mul(y, y, t[:])
                if with_bias:
                    # std = var * rstd
                    nc.gpsimd.tensor_mul(st[:, 12:16], st[:, 8:12], y)

            def emit_stsb_head(c):
                """mean row [1, 512] at partition 0 (matmul operands must
                sit at base partition 0): bf16 per-column transposes"""
                st = stats[c]
                if not with_bias:
                    stb = smallp.tile([128, 4], BF16, name=f"stb_{c}",
                                      tag="stb", bufs=2)
                    nc.vector.tensor_copy(stb[:], st[:, 0:4])
                    u_ps = psM.tile([128, 512], F32, name=f"u_{c}", tag="m")
                    ub = u_ps.bitcast(BF16)
                    for i in range(4):
                        nc.tensor.transpose(ub[0:1, i * 128:(i + 1) * 128],
                                            stb[:, i:i + 1], identb_sb)
                    row = smallp.tile([1, 512], BF16, name=f"row_{c}",
                                      tag="mrow", bufs=4)
                    nc.vector.tensor_copy(row[0:1, :], ub[0:1, 0:512])
                    mrow[c] = row[0:1, 0:512]
                    return
                u_ps = psM.tile([128, 512], F32, name=f"u_{c}", tag="m")
                for i in range(4):
                    nc.tensor.transpose(u_ps[0:1, i * 128:(i + 1) * 128],
                                        st[:, i:i + 1], identf_sb)
                row = smallp.tile([1, 512], BF16, name=f"row_{c}",
                                  tag="mrow", bufs=4)
                nc.vector.tensor_copy(row[0:1, :], u_ps[0:1, 0:512])
                mrow[c] = row[0:1, 0:512]
                if with_bias:
                    # transpose outputs must land on partition 0 (HW rule)
                    u2 = psM.tile([128, 512], F32, name=f"u2_{c}", tag="m")
                    for i in range(4):
                        nc.tensor.transpose(
                            u2[0:1, i * 128:(i + 1) * 128],
                            st[:, 12 + i:13 + i], identf_sb)
                    dr = smallp.tile([1, 512], BF16, name=f"dr_{c}",
                                     tag="drow", bufs=4)
                    nc.vector.tensor_copy(dr[0:1, :], u2[0:1, 0:512])
                    drow[c] = dr

            def emit_vtail(hp, c, v_a, v_b):
                """v rank1 (needs mean rows) + evict with per-partition rstd"""
                st = stats[c]
                for half, v_ps in ((0, v_a), (1, v_b)):
                    for li in range(2):
                        i = half * 2 + li
                        last = (li == 1)
                        nc.tensor.matmul(v_ps[:, li * 130:li * 130 + 130],
                                         mrow[c][:, i * 128:(i + 1) * 128],
                                         auxv(hp, 2),
                                         start=False,
                                         stop=last and not with_bias,
                                         skip_group_check=True)
                        if with_bias:
                            nc.tensor.matmul(v_ps[:, li * 130:li * 130 + 130],
                                             drow[c][0:1,
                                                     i * 128:(i + 1) * 128],
                                             biasv(hp, 2),
                                             start=False, stop=last,
                                             skip_group_check=True)
                    for li in range(2):
                        i = half * 2 + li
                        jb = (c * 4 + i) * 130
                        dst = v_nat[hp][:, jb:jb + 130].rearrange(
                            "p (h c) -> p h c", c=65)[:, :, 0:64]
                        nc.vector.tensor_scalar(
                            out=dst,
                            in0=v_ps[:, li * 130:li * 130 + 128].rearrange(
                                "p (h c) -> p h c", c=64),
                            scalar1=st[:, 4 + i:5 + i], scalar2=None,
                            op0=ALU.mult)

            qk_pr = {}; qk_qn = {}

            def emit_c0_ktmajor():
                """chunk 0 of pass 0 with all accumulations advancing
                k-tile-major, so PE work tracks the xt arrival order; the
                gram runs in bf16 off xt (xf8 lands after the first half)"""
                g_ps = psM.tile([128, 512], F32, name="g_0", tag="m")
                va = psM.tile([128, 260], F32, name="v0_0_0", tag="m")
                vb = psM.tile([128, 260], F32, name="v0_0_1", tag="m")
                prq = psS.tile([128, 512], F32, name="p00_0", tag="S")
                prk = psS.tile([128, 512], F32, name="p10_0", tag="S")
                for kt in range(KT):
                    for i in range(4):
                        t0 = i * 128
                        for g2 in range(2):
                            nc.tensor.matmul(
                                g_ps[:, (i * 2 + g2) * 64:
                                     (i * 2 + g2 + 1) * 64],
                                xtv(kt, t0, t0 + 128),
                                xtv(kt, t0 + g2 * 64, t0 + g2 * 64 + 64),
                                start=(kt == 0 and i == 0 and g2 == 0),
                                stop=(kt == KT - 1 and i == 3 and g2 == 1),
                                skip_group_check=True)
                    for half, v_ps in ((0, va), (1, vb)):
                        for li in range(2):
                            i = half * 2 + li
                            nc.tensor.matmul(
                                v_ps[:, li * 130:li * 130 + 130],
                                xtv(kt, i * 128, (i + 1) * 128),
                                wvv(kt, 0),
                                start=(kt == 0 and li == 0),
                                stop=False, skip_group_check=True)
                    for i in range(4):
                        nc.tensor.matmul(
                            prq[:, i * 128:(i + 1) * 128],
                            xtv(kt, i * 128, (i + 1) * 128),
                            wqkv(kt, 0, 0),
                            start=(kt == 0 and i == 0), stop=False,
                            skip_group_check=True)
                    if with_bias:
                        for i in range(4):
                            nc.tensor.matmul(
                                prk[:, i * 128:(i + 1) * 128],
                                xtv(kt, i * 128, (i + 1) * 128),
                                wqkv(kt, 0, 1),
                                start=(kt == 0 and i == 0), stop=False,
                                skip_group_check=True)
                    else:
                        nc.tensor.matmul(
                            prk[:], wqkv(kt, 0, 1), xtv(kt, 0, 512),
                            start=(kt == 0), stop=False,
                            skip_group_check=True)
                qk_pr[0, 0, 0] = prq
                qk_pr[0, 0, 1] = prk
                return g_ps, va, vb

            def emit_qk_mm(hp, c, which, half):
                """natural-layout projection matmuls (psS ring; no stats
                deps). half 0 = token tiles 0-1, half 1 = tiles 2-3"""
                if half == 0:
                    pr = psS.tile([128, 512], F32, name=f"p{which}{hp}_{c}",
                                  tag="S")
                    qk_pr[hp, c, which] = pr
                else:
                    pr = qk_pr[hp, c, which]
                for i in (0, 1) if half == 0 else (2, 3):
                    t0 = c * 512 + i * 128
                    for kt in range(KT):
                        nc.tensor.matmul(
                            pr[:, i * 128:(i + 1) * 128],
                            xtv(kt, t0, t0 + 128),
                            wqkv(kt, hp, which),
                            start=(i == 0 and kt == 0), stop=False,
                            skip_group_check=True)

            def emit_qk_fin(hp, c, which):
                """rank-1 LN mean correction + per-partition rstd evict"""
                st = stats[c]
                pr = qk_pr[hp, c, which]
                for i in range(4):
                    last = (i == 3)
                    nc.tensor.matmul(pr[:, i * 128:(i + 1) * 128],
                                     mrow[c][:, i * 128:(i + 1) * 128],
                                     auxv(hp, which),
                                     start=False,
                                     stop=last and not with_bias,
                                     skip_group_check=True)
                    if with_bias:
                        nc.tensor.matmul(pr[:, i * 128:(i + 1) * 128],
                                         drow[c][0:1, i * 128:(i + 1) * 128],
                                         biasv(hp, which),
                                         start=False, stop=last,
                                         skip_group_check=True)
                qn = smallp.tile([128, 512], BF16, name=f"qn{which}{hp}_{c}",
                                 tag=f"qn{which}", bufs=2)
                for i in range(4):
                    # 2 on ACT: the q transpose waits on these and the DVE
                    # queue is the longer one in phase-A windows
                    if i < 2:
                        nc.scalar.mul(qn[:, i * 128:(i + 1) * 128],
                                      pr[:, i * 128:(i + 1) * 128],
                                      st[:, 4 + i:5 + i])
                    else:
                        nc.vector.tensor_scalar(
                            out=qn[:, i * 128:(i + 1) * 128],
                            in0=pr[:, i * 128:(i + 1) * 128],
                            scalar1=st[:, 4 + i:5 + i], scalar2=None,
                            op0=ALU.mult)
                qk_qn[hp, c, which] = qn

            def emit_qk_tr(hp, c, which):
                """transpose natural [t, hd] tiles into qT/kT"""
                qn = qk_qn[hp, c, which]
                tr = psM.tile([128, 512], BF16, name=f"tr{which}{hp}_{c}",
                              tag="m")
                for i in range(4):
                    nc.tensor.transpose(tr[:, i * 128:(i + 1) * 128],
                                        qn[:, i * 128:(i + 1) * 128],
                                        identb_sb)
                dst = qT[hp] if which == 0 else kTt[hp]
                nc.vector.tensor_copy(dst[:, c * 512:(c + 1) * 512], tr[:])

            def emit_k_mm(hp, c, half):
                """k projected directly transposed: out [hd, t] with the
                weight tile stationary; no separate PE transpose, and the
                per-token rstd is deferred to the exp's per-partition scale"""
                if half == 0:
                    pr = psS.tile([128, 512], F32, name=f"pk{hp}_{c}",
                                  tag="S")
                    qk_pr[hp, c, 1] = pr
                else:
                    pr = qk_pr[hp, c, 1]
                t0 = c * 512
                for kt in range(KT // 2 * half, KT // 2 * (half + 1)):
                    nc.tensor.matmul(
                        pr[:], wqkv(kt, hp, 1), xtv(kt, t0, t0 + 512),
                        start=(kt == 0), stop=False, skip_group_check=True)

            def emit_k_fin(hp, c):
                """rank-1 LN mean correction (outer product via K=1 matmul)
                + plain bf16 eviction into kT"""
                pr = qk_pr[hp, c, 1]
                for i in range(4):
                    nc.tensor.matmul(pr[:, i * 128:(i + 1) * 128],
                                     auxv(hp, 1),
                                     mrow[c][:, i * 128:(i + 1) * 128],
                                     start=False, stop=(i == 3),
                                     skip_group_check=True)
                nc.vector.tensor_copy(kTt[hp][:, c * 512:(c + 1) * 512],
                                      pr[:])

            def gen_A(hp):
                """generator emitting phase A in fine-grained steps; yields
                the PE-cycle estimate of what it just emitted (drive uses it
                to budget filler insertion), or ("ready", hp, c) markers.
                Pass 0 also produces the shared LN statistics."""
                if hp == 0:
                    g, va, vb = emit_c0_ktmajor()
                    emit_diag(0, g)
                    yield 2080
                else:
                    va = emit_vproj(1, 0, 0)
                    yield 2080
                    vb = emit_vproj(1, 0, 1)
                    yield 2080
                for c in range(NCH):
                    if hp == 0:
                        emit_meanvar(c, va, vb)
                        yield 0
                        if STAGE < 3:
                            if c + 1 < NCH:
                                g = emit_gram(c + 1)
                                emit_diag(c + 1, g)
                                va = emit_vproj(0, c + 1, 0)
                                vb = emit_vproj(0, c + 1, 1)
                            continue
                        emit_stsb_head(c)
                        yield 256
                    if not (hp == 0 and c == 0):
                        emit_qk_mm(hp, c, 0, 0)
                        yield 2048
                        emit_qk_mm(hp, c, 0, 1)
                    yield 2048
                    emit_vtail(hp, c, va, vb)
                    yield 520
                    emit_qk_fin(hp, c, 0)
                    yield 1024
                    if with_bias:
                        if not (hp == 0 and c == 0):
                            emit_qk_mm(hp, c, 1, 0)
                            yield 2048
                            emit_qk_mm(hp, c, 1, 1)
                            yield 2048
                        emit_qk_fin(hp, c, 1)
                        yield 1024
                        emit_qk_tr(hp, c, 0)
                        yield 512
                        emit_qk_tr(hp, c, 1)
                        yield ("ready", hp, c)
                    else:
                        if not (hp == 0 and c == 0):
                            emit_k_mm(hp, c, 0)
                            yield 2048
                            emit_k_mm(hp, c, 1)
                            yield 2048
                        emit_k_fin(hp, c)
                        yield 1024
                        emit_qk_tr(hp, c, 0)
                        yield ("ready", hp, c)
                    if c + 1 < NCH:
                        if hp == 0:
                            g = emit_gram(c + 1)
                            emit_diag(c + 1, g)
                            yield 1024
                        va = emit_vproj(hp, c + 1, 0)
                        yield 2080
                        vb = emit_vproj(hp, c + 1, 1)
                        yield 2080

            # =============== phase B (attention) ===============
            def jt_off(c4, jt):
                return 0 if jt < 4 * c4 else (jt - 4 * c4) * 128

            def emit_sblk(hp, c4, jt, h):
                """S block for one (jt, head); exp; diag mask."""
                o = jt_off(c4, jt)
                w = 512 - o
                sp = psS.tile([128, 512], F32, name=f"s{hp}{c4}{jt}{h}",
                              tag="S")
                nc.tensor.matmul(
                    sp[:, 0:w],
                    kTt[hp][h * 64:(h + 1) * 64, jt * 128:(jt + 1) * 128],
                    qT[hp][h * 64:(h + 1) * 64, c4 * 512 + o:(c4 + 1) * 512],
                    start=True, stop=True)
                p = ppool.tile([128, 512], BF16, name=f"e{hp}{c4}{jt}{h}",
                               tag="p")
                # kT is evicted unscaled; its per-key rstd rides the exp's
                # per-partition scale (keys sit on partitions here)
                sc = (1.0 if with_bias
                      else stats[jt // 4][:, 4 + jt % 4:5 + jt % 4])
                nc.scalar.activation(p[:, 0:w], sp[:, 0:w], AF.Exp, scale=sc)
                if jt >= 4 * c4:   # diagonal block: mask first 128 cols
                    # p is SBUF so GPSIMD may touch it; keeps DVE clear
                    nc.gpsimd.tensor_mul(p[:, 0:128], p[:, 0:128], tri_sb[:])
                return p, o

            def emit_pv(hp, c4, at_ps, p, jt, o, h):
                ils = [il for il in range(4) if 4 * c4 + il >= jt]
                if jt >= 4 * c4 and jt != 0 and len(ils) > 1:
                    # masked tile last (jt==0 keeps order: its il0/il2 writes
                    # carry the start flags that mark the psum banks)
                    ils = ils[1:] + ils[:1]
                for il in ils:
                    it = 4 * c4 + il
                    lo = il * 128 - o
                    abase = (il % 2) * 130 + (il // 2) * 512 + h * 65
                    nc.tensor.matmul(
                        at_ps[:, abase:abase + 65],
                        p[:, lo:lo + 128],
                        v_nat[hp][:, jt * 130 + h * 65:
                                  jt * 130 + h * 65 + 65],
                        start=(jt == 0 and h == 0 and il % 2 == 0),
                        stop=(jt == it),
                        skip_group_check=True)

            attn_an = {}

            def _at_views(at_ps):
                """strided views of at_ps [128, 1024] as [p, a, b, h, c]
                with col = 512a + 130b + 65h + c  (il = 2a + b)"""
                v = at_ps.rearrange("p (a r) -> p a r", a=2)[:, :, 0:260] \
                    .rearrange("p a (b r) -> p a b r", b=2) \
                    .rearrange("p a b (h c) -> p a b h c", h=2)
                return v[:, :, :, :, 0:64], v[:, :, :, :, 64:65]

            def emit_norm_il(hp, c4, at_ps, il):
                """softmax normalize one query tile (DVE)"""
                abase = (il % 2) * 130 + (il // 2) * 512
                rcp = smallp.tile([128, 2], F32, name=f"rc{hp}{c4}{il}",
                                  tag="rcp", bufs=4)
                nc.vector.reciprocal(rcp[:, 0:1],
                                     at_ps[:, abase + 64:abase + 65])
                nc.vector.reciprocal(rcp[:, 1:2],
                                     at_ps[:, abase + 129:abase + 130])
                an = smallp.tile([128, 128], BF16, name=f"an{hp}{c4}{il}",
                                 tag="an", bufs=4)
                for h in range(2):
                    nc.vector.tensor_scalar(
                        out=an[:, h * 64:(h + 1) * 64],
                        in0=at_ps[:, abase + h * 65:abase + h * 65 + 64],
                        scalar1=rcp[:, h:h + 1], scalar2=None,
                        op0=ALU.mult)
                attn_an[hp, c4, il] = an

            def emit_attn_norm(hp, c4, at_ps):
                """fused normalize: ONE reciprocal over the 8 denominator
                columns + ONE broadcast multiply over all 512 attention
                columns (vs 8 + 8 per-il DVE instructions)"""
                pv, dv = _at_views(at_ps)
                rcp = smallp.tile([128, 8], F32, name=f"rcf{hp}{c4}",
                                  tag="rcpf", bufs=2)
                rcv = rcp.rearrange("p (a b h c) -> p a b h c", a=2, b=2, h=2)
                nc.vector.reciprocal(rcv, dv)
                an = smallp.tile([128, 512], BF16, name=f"anf{hp}{c4}",
                                 tag="anf", bufs=2)
                anv = an.rearrange("p (a b h c) -> p a b h c", a=2, b=2, h=2)
                nc.vector.tensor_tensor(
                    anv, pv, rcv.broadcast_to([128, 2, 2, 2, 64]), ALU.mult)
                for il in range(4):
                    attn_an[hp, c4, il] = an[:, il * 128:(il + 1) * 128]

            def emit_attn_tr(hp, c4):
                """transpose normalized tiles into attnT"""
                tr_ps = psM.tile([128, 512], BF16, name=f"tr{hp}{c4}",
                                 tag="m")
                for il in range(4):
                    nc.tensor.transpose(tr_ps[:, il * 128:(il + 1) * 128],
                                        attn_an[hp, c4, il][:], identb_sb)
                nc.vector.tensor_copy(attnT[hp][:, c4 * 512:(c4 + 1) * 512],
                                      tr_ps[:])

            def emit_outproj(it, eng_pick, on_act=False):
                """y tile: BOTH head-pairs' contributions accumulate in the
                same PSUM before one eviction + one DMA (emitted only once
                pass 1's attnT for this tile exists)"""
                y_sb = smallp.tile([128, D], BF16, name=f"ys_{it}",
                                   tag="ysb", bufs=3)
                for e in range(2):
                    y_ps = psM.tile([128, 512], F32, name=f"y{it}_{e}",
                                    tag="m")
                    for hp in range(2):
                        nc.tensor.matmul(
                            y_ps[:],
                            attnT[hp][:, it * 128:(it + 1) * 128],
                            wo_sb[:, hp * D + e * 512:hp * D + (e + 1) * 512],
                            start=(hp == 0), stop=(hp == 1),
                            skip_group_check=True)
                    # GPSIMD cannot read PSUM; evictions live on DVE so ACT
                    # stays exp-only -- except the drain tail, where the
                    # exps are done and DVE is the backlogged queue
                    if on_act:
                        nc.scalar.copy(y_sb[:, e * 512:(e + 1) * 512],
                                       y_ps[:])
                    else:
                        nc.vector.tensor_copy(
                            y_sb[:, e * 512:(e + 1) * 512], y_ps[:])
                    # per-half DMA: the first half ships while the second
                    # half is still evicting (shortens the drain tail)
                    nc.sync.dma_start(
                        y_d.ap()[it * 128:(it + 1) * 128,
                                 e * 512:(e + 1) * 512],
                        y_sb[:, e * 512:(e + 1) * 512])

            def gen_B(hp, deferred, late):
                for c4 in range(NCH):
                    yield ("need", hp, c4)
                    njt = 4 * c4 + 4
                    at_ps = psA.tile([128, 1024], F32, name=f"at{hp}{c4}",
                                     tag="attn")
                    last_unit = (STAGE >= 5 and hp == 1 and c4 == NCH - 1)
                    tr_last = [None]

                    def stream_il(il, hp=hp, c4=c4, at_ps=at_ps,
                                  tr_last=tr_last):
                        """last chunk streams per-tile finish + outproj so
                        the tail drains early"""
                        if tr_last[0] is None:
                            tr_last[0] = psM.tile([128, 512], BF16,
                                                  name=f"trL{hp}{c4}",
                                                  tag="m")
                        emit_norm_il(hp, c4, at_ps, il)
                        nc.tensor.transpose(
                            tr_last[0][:, il * 128:(il + 1) * 128],
                            attn_an[hp, c4, il][:], identb_sb)
                        it = 4 * c4 + il
                        nc.vector.tensor_copy(
                            attnT[hp][:, it * 128:(it + 1) * 128],
                            tr_last[0][:, il * 128:(il + 1) * 128])
                        if STAGE >= 6:
                            emit_outproj(it, il, on_act=True)

                    # PV lags its S by one jt for h0 and two jts for h1:
                    # the h1 exp lands ~1.4us after its S pair, which is
                    # more than one jt of PE work, so a 1-jt lag alone
                    # stalls the in-order PE queue on the h1 PVs.
                    pend = []   # (p, jt, o, h) awaiting PV emission
                    for jt in range(njt):
                        cur = []
                        for h in range(2):
                            p, o = emit_sblk(hp, c4, jt, h)
                            cur.append((p, jt, o, h))
                        # ask drive to insert ~FILL_CYC PE-cycles of filler
                        # (deferred + phase-A steps) so the PV matmuls below
                        # don't reach the head of the in-order PE queue
                        # before their exp has landed
                        yield ("fill", FILL_CYC)
                        if STAGE >= 5:
                            emitted_il = None
                            for (p, j, o, h) in list(pend):
                                if j == jt - 1:
                                    continue   # both heads lag two jts
                                emit_pv(hp, c4, at_ps, p, j, o, h)
                                pend.remove((p, j, o, h))
                                if h == 1 and last_unit and j >= 4 * c4:
                                    emitted_il = j - 4 * c4
                            if emitted_il is not None:
                                stream_il(emitted_il)
                        pend.extend(cur)
                    if STAGE >= 5:
                        for (p, j, o, h) in pend:
                            emit_pv(hp, c4, at_ps, p, j, o, h)
                            if (h == 1 and last_unit and j >= 4 * c4
                                    and j - 4 * c4 < 3):
                                stream_il(j - 4 * c4)
                        if last_unit:
                            stream_il(3)
                        else:
                            deferred.append((0,
                                lambda hp=hp, c4=c4, at=at_ps:
                                emit_attn_norm(hp, c4, at)))
                            deferred.append((512,
                                lambda hp=hp, c4=c4: emit_attn_tr(hp, c4)))
                            if STAGE >= 6 and hp == 1:
                                # y needs both passes' attnT: emit the first
                                # three chunks' worth during pass-1 c4s
                                for il in range(4):
                                    deferred.append((2048,
                                        lambda it=4 * c4 + il, il=il:
                                        emit_outproj(it, il)))
                                if c4 == NCH - 2:
                                    # pass-0 tiles beyond pass-1's c4 range
                                    # never get a c4 of their own; none
                                    # exist (ranges match), so nothing here
                                    pass

            # =============== master schedule ===============
            def chain(*gens):
                for g in gens:
                    yield from g

            def drive(bgen, agen, deferred):
                """interleave B with budgeted filler: on each B ("fill", n)
                request, insert deferred items and phase-A steps totalling
                ~n PE-cycles. Never let B emit reads of phase-A tiles before
                their writers exist: B announces ("need", hp, c4); A
                announces ("ready", hp, c)."""
                ready = set()
                a_done = [False]

                def pump_a():
                    """advance A one step; returns its PE-cycle estimate"""
                    if a_done[0]:
                        return 0
                    try:
                        item = next(agen)
                    except StopIteration:
                        a_done[0] = True
                        return 0
                    if isinstance(item, tuple):
                        ready.add(item[1:])
                        return 0
                    return item or 0

                while True:
                    try:
                        item = next(bgen)
                    except StopIteration:
                        break
                    if item is not None and item[0] == "need":
                        while item[1:] not in ready and not a_done[0]:
                            pump_a()
                            if deferred:
                                deferred.pop(0)[1]()
                        assert item[1:] in ready, f"A never produced {item}"
                    else:
                        need = item[1] if item else 0
                        while need > 0:
                            if deferred:
                                cyc, fn = deferred.pop(0)
                                fn()
                                need -= max(cyc, 64)
                            if need > 0 and not a_done[0]:
                                need -= max(pump_a(), 64)
                            elif not deferred:
                                break
                while not a_done[0]:
                    pump_a()

            deferred = []
            late = []
            if STAGE >= 4:
                aq = chain(gen_A(0), gen_A(1))
                bq = chain(gen_B(0, deferred, late),
                           gen_B(1, deferred, late))
                drive(bq, aq, deferred)
                while deferred:
                    deferred.pop(0)[1]()
                while late:
                    late.pop(0)[1]()
            else:
                for _ in chain(gen_A(0), gen_A(1)):
                    pass

    nc.compile()
    return nc


_PROG_CACHE = {}


def _get_program(with_bias):
    key = (with_bias, STAGE)
    if key not in _PROG_CACHE:
        _PROG_CACHE[key] = _build_program(with_bias)
    return _PROG_CACHE[key]


def kernel(x, ln_g, ln_b, lnc_g, lnc_b, Wq, Wkv, Wo):
    global LAST_RESULTS
    x = np.ascontiguousarray(np.asarray(x, dtype=np.float32))
    ln_g = np.asarray(ln_g, np.float32); ln_b = np.asarray(ln_b, np.float32)
    lnc_g = np.asarray(lnc_g, np.float32); lnc_b = np.asarray(lnc_b, np.float32)
    Wq = np.asarray(Wq, np.float32); Wkv = np.asarray(Wkv, np.float32)
    Wo = np.asarray(Wo, np.float32)
    scale = DH ** -0.5

    with_bias = bool(np.any(ln_b) or np.any(lnc_b))
    nc = _get_program(with_bias)

    xt_f = np.ascontiguousarray(np.transpose(x, (0, 2, 1)))   # [B, D, N]
    xt = xt_f.astype(NPBF)
    # [b, p, kt*N + t] = x^T[b, kt*128+p, t], fp8 e4m3 for the DoubleRow gram
    xf8 = np.ascontiguousarray(
        xt_f.reshape(B, KT, 128, N).transpose(0, 2, 1, 3)
        .reshape(B, 128, KT * N)).astype(ml_dtypes.float8_e4m3)
    tri = np.triu(np.ones((128, 128), np.float32)).astype(NPBF)
    identb = np.eye(128, dtype=np.float32).astype(NPBF)
    identf = np.eye(128, dtype=np.float32)

    in_maps = []
    for c in range(NCORES):
        b = c // (NCORES // B)            # batch of this core
        g = c % (NCORES // B)             # head-group (4 heads)
        cs = slice(g * HG * DH, (g + 1) * HG * DH)
        Wq_eff = ln_g[:, None] * Wq[:, cs] * scale          # [D, 256]
        Wk_eff = lnc_g[:, None] * Wkv[:, :H * DH][:, cs]
        Wv_eff = lnc_g[:, None] * Wkv[:, H * DH:][:, cs]
        # per k-tile: [q_hp0 | k_hp0 | q_hp1 | k_hp1] (128 each)
        wqk = np.concatenate([Wq_eff[:, 0:128], Wk_eff[:, 0:128],
                              Wq_eff[:, 128:256], Wk_eff[:, 128:256]], axis=1)
        wqk = np.ascontiguousarray(
            wqk.reshape(KT, 128, 512).transpose(1, 0, 2).reshape(128, KT * 512))
        # per k-tile: [v_hp0 128 | 1/D | pad | v_hp1 128 | 1/D | pad]
        oneD = np.full((D, 1), 1.0 / D)
        zD = np.zeros((D, 1), np.float32)
        wv = np.concatenate([Wv_eff[:, 0:128], oneD, zD,
                             Wv_eff[:, 128:256], oneD, zD], axis=1)
        wv = np.ascontiguousarray(
            wv.reshape(KT, 128, 260).transpose(1, 0, 2).reshape(128, KT * 260))
        Wo_cs = Wo[cs, :]                                    # [256, D]
        wo = np.concatenate([Wo_cs[0:128, :], Wo_cs[128:256, :]], axis=1)
        aux = np.zeros((1, 1280), np.float32)
        for hp in range(2):
            hs = slice(hp * 128, (hp + 1) * 128)
            aux[0, hp * 640 + 0:hp * 640 + 128] = -Wq_eff[:, hs].sum(0)
            aux[0, hp * 640 + 128:hp * 640 + 256] = -Wk_eff[:, hs].sum(0)
            aux[0, hp * 640 + 256:hp * 640 + 384] = -Wv_eff[:, hs].sum(0)
            aux[0, hp * 640 + 512:hp * 640 + 640] = 1.0
        m = {
            "xt": xt[b],
            "xf8": xf8[b],
            "wqk": wqk.astype(NPBF),
            "wv": wv.astype(NPBF),
            "wo": np.ascontiguousarray(wo).astype(NPBF),
            "aux": aux.astype(NPBF),
            "tri": tri, "identb": identb, "identf": identf,
        }
        if with_bias:
            br = np.zeros((1, 772), np.float32)
            for hp in range(2):
                hs = slice(g * HG * DH + hp * 128, g * HG * DH + (hp + 1) * 128)
                br[0, hp * 386 + 0:hp * 386 + 128] = ln_b @ Wq[:, hs] * scale
                br[0, hp * 386 + 128:hp * 386 + 256] = \
                    lnc_b @ Wkv[:, :H * DH][:, hs]
                br[0, hp * 386 + 256:hp * 386 + 384] = \
                    lnc_b @ Wkv[:, H * DH:][:, hs]
            m["biasr"] = br.astype(NPBF)
        in_maps.append(m)

    res = run_bass_kernel_spmd(nc, in_maps, core_ids=list(range(NCORES)),
                               trace=TRACE, **TRACE_KWARGS)
    LAST_RESULTS = res
    ys = []
    gpb = NCORES // B
    for b in range(B):
        yb = res.results[b * gpb]["y"].astype(np.float32)
        for g in range(1, gpb):
            yb += res.results[b * gpb + g]["y"].astype(np.float32)
        ys.append(yb)
    return np.stack(ys)


# revision 91
# speedup vs baseline: 1.0002x; 1.0002x over previous
"""Trainium2 Bass kernel for nn_Attention_85005992722686.

Batch x head-group sharded causal attention over 8 NeuronCores.
Core c owns batch c//4 and heads {4g..4g+3} (g = c%4), processed as two
head-pair passes hp in {0,1} (the same two-pass pipeline shape a
2-batch schedule would use, so PSUM stays at 8 banks).  Splitting the
batch across the core grid halves the per-core x / fp8-x / y DMA bytes
(the cost model serializes all DMA on one device, so bytes ~= wall
time) and computes the LN statistics once instead of per pass.

Both layernorms fold into the projection weights (gamma scales the
weight columns, the mean term becomes a rank-1 PSUM correction, rstd a
per-token scale); per-core partial outputs through the row-shard of Wo
are summed on the host within each batch's 4-core group.

All matmul operands are bf16 (PSUM accumulates fp32) except the token
gram for the LN variance, which runs in fp8e4m3 DoubleRow (0.5 cyc/row,
256-deep contraction; variance averages 1024 squares so the fp8 noise
is ~0.2%).  The cost model charges a matmul `out_free_size` rows at
0.4167ns/row regardless of K and M, so the structure minimizes total
streamed output columns:

  phase A (per 512-token chunk):
    gram:    64-col token-gram diag blocks; diag = sum(x^2) per token
    v-proj:  natural layout out[t, 130] = [v_h0|v_h1|mean|pad]; the 1/D
             weight column yields token means for free
    q-proj:  natural layout [t, 128] + rank-1 mean fix, rstd applied as
             a per-partition scalar at eviction, PE-transposed to [hd,t]
    k-proj:  directly transposed (weights stationary, x moving); its
             per-token rstd rides the exp's per-partition scale
    rstd:    Newton rsqrt on GPSIMD (var is ~1, three mult/add steps)
    mean row: per-column PE transposes landing on partition 0 (no DMA)
  phase B (per 512-query chunk c4):
    S^T blocks [j, i] per (jt, head) -> exp on ACT (scale = rstd_k) ->
    tri-mask (diag) on GPSIMD
    PV in natural orientation: out[i, 65] = P-block^T @ [v_h|1], PSUM-
    accumulated over jt; col 64 is the softmax denominator; PV lags its
    S by 1 jt (h0) / 2 jts (h1) to cover the exp latency
    normalize fused (one reciprocal + one broadcast multiply per chunk)
    transpose; y = sum_hp attnT_hp^T @ Wo_hp accumulated in PSUM

A dependency-paced interleaver merges both passes' phase A and B
emission (B announces chunk needs and filler budgets, A announces chunk
completion and PE-cycle estimates) so the PE stream stays dense while
ACT digests the exps; PSUM: 3 banks S ring, 2 banks attention
accumulators, 3 banks misc ring.
"""
import sys
sys.path.insert(0, '/opt/trn_rl_repo')
import numpy as np
import ml_dtypes
import concourse.bass as bass
import concourse.bacc as bacc
import concourse.tile as tile
from concourse import mybir
from concourse.bass_utils import run_bass_kernel_spmd

F32 = mybir.dt.float32
BF16 = mybir.dt.bfloat16
FP8 = mybir.dt.float8e4
DR = mybir.MatmulPerfMode.DoubleRow
AF = mybir.ActivationFunctionType
ALU = mybir.AluOpType

B, N, D = 2, 2048, 1024
H, DH = 16, 64
EPS = 1e-5
NCORES = 8
HG = 4            # heads per core (2 head-pairs x 2 heads)
KT = D // 128     # 8 k-tiles over model dim
NT = N // 128     # 16 token tiles
NCH = N // 512    # 4 chunks of 512 tokens

STAGE = 6         # debug: 2 gram/v/stats, 3 full phase A, 4 +S/exp, 5 +PV, 6 full
FILL_CYC = 1024   # PE-cycles of deferred filler inserted after each S pair
TRACE = False
TRACE_KWARGS = {}
LAST_RESULTS = None
NPBF = ml_dtypes.bfloat16


def _build_program(with_bias):
    nc = bacc.Bacc("TRN2", target_bir_lowering=False, debug=False,
                   num_devices=NCORES)
    # ---------------- dram io (one batch, 4 heads per core) ----------------
    xt_d = nc.dram_tensor("xt", [D, N], BF16, kind="ExternalInput")
    # fp8 copy of x^T, k-tiles side by side: [p, kt*N + t] = x[kt*128+p, t]
    xf8_d = nc.dram_tensor("xf8", [128, KT * N], FP8, kind="ExternalInput")
    # host-packed per k-tile: [q_hp0 128 | k_hp0 128 | q_hp1 128 | k_hp1 128]
    wqk_d = nc.dram_tensor("wqk", [128, KT * 512], BF16, kind="ExternalInput")
    # per k-tile: [v_hp0 130 | v_hp1 130]
    wv_d = nc.dram_tensor("wv", [128, KT * 260], BF16, kind="ExternalInput")
    # [wo_hp0 D | wo_hp1 D]
    wo_d = nc.dram_tensor("wo", [128, 2 * D], BF16, kind="ExternalInput")
    # per hp at hp*640: [ncs_q 0:128 | ncs_k 128:256 | ncs_v 256:386 | ones 512:640]
    aux_d = nc.dram_tensor("aux", [1, 1280], BF16, kind="ExternalInput")
    tri_d = nc.dram_tensor("tri", [128, 128], BF16, kind="ExternalInput")
    identb_d = nc.dram_tensor("identb", [128, 128], BF16, kind="ExternalInput")
    identf_d = nc.dram_tensor("identf", [128, 128], F32, kind="ExternalInput")
    if with_bias:
        # per hp at hp*386: [bq 0:128 | bk 128:256 | bv 256:321]
        biasr_d = nc.dram_tensor("biasr", [1, 772], BF16, kind="ExternalInput")
    y_d = nc.dram_tensor("y", [N, D], BF16, kind="ExternalOutput")

    with tile.TileContext(nc) as tc:
        with tc.tile_pool(name="wpool", bufs=1) as wpool, \
             tc.tile_pool(name="xpool", bufs=2) as xpool, \
             tc.tile_pool(name="big", bufs=2) as bigp, \
             tc.tile_pool(name="small", bufs=1) as smallp, \
             tc.tile_pool(name="ppool", bufs=14) as ppool, \
             tc.tile_pool(name="psS", bufs=3, space="PSUM") as psS, \
             tc.tile_pool(name="psA", bufs=1, space="PSUM") as psA, \
             tc.tile_pool(name="psM", bufs=3, space="PSUM") as psM:

            # ---- input DMAs: half tiles pace chunk 0; xf8 (only needed
            # from chunk 1's gram on) slots in after the first half.
            xt_sb = {}
            xf8_sb = [None]

            def load_x():
                # three waves: chunk-0 quarters (all chunk 0 needs), then
                # chunk-1 quarters, the fp8 copy, then the back half --
                # chunk 0's whole chain starts ~3us earlier this way
                for wave, lo, hi in ((0, 0, 512), (1, 512, 1024),
                                     (3, 1024, 2048)):
                    for kt in range(KT):
                        t = xpool.tile([128, hi - lo], BF16,
                                       name=f"x_{kt}_{wave}",
                                       tag=f"bx{kt}w{wave}", bufs=1)
                        nc.sync.dma_start(
                            t[:], xt_d.ap()[kt * 128:(kt + 1) * 128, lo:hi])
                        xt_sb[kt, wave] = t
                # xf8 last: chunk-1 grams in bf16 off waves 0-1, so only
                # chunk 2-3 grams (~24us+) need the fp8 copy, and the back
                # half of x unblocks chunks 2-3's projections ~6us earlier
                t8 = xpool.tile([128, KT * N], FP8, name="xf8",
                                tag="xf8", bufs=1)
                xf8_sb[0] = t8.rearrange("p (k t) -> p k t", t=N)
                for s in range(4):
                    w = KT * N // 4
                    nc.sync.dma_start(t8[:, s * w:(s + 1) * w],
                                      xf8_d.ap()[:, s * w:(s + 1) * w])

            identf_sb = wpool.tile([128, 128], F32, name="identf_sb")
            nc.scalar.dma_start(identf_sb[:], identf_d.ap()[:, :])
            wv_sb = wpool.tile([128, KT * 260], BF16, name="wv_sb")
            nc.scalar.dma_start(wv_sb[:], wv_d.ap()[:, :])
            load_x()
            wqk_sb = wpool.tile([128, KT * 512], BF16, name="wqk_sb")
            nc.scalar.dma_start(wqk_sb[:], wqk_d.ap()[:, :])
            aux_sb = wpool.tile([1, 1280], BF16, name="aux_sb")
            nc.scalar.dma_start(aux_sb[:], aux_d.ap()[:, :])
            identb_sb = wpool.tile([128, 128], BF16, name="identb_sb")
            nc.scalar.dma_start(identb_sb[:], identb_d.ap()[:, :])
            # tri (first diag exp ~17us) and wo (first outproj ~25us) ride
            # the sync queue BEHIND the x stream so their wire time doesn't
            # compete with the startup waves on the serialized DMA device
            tri_sb = wpool.tile([128, 128], BF16, name="tri_sb")
            nc.sync.dma_start(tri_sb[:], tri_d.ap()[:, :])
            wo_sb = wpool.tile([128, 2 * D], BF16, name="wo_sb")
            nc.sync.dma_start(wo_sb[:], wo_d.ap()[:, :])
            if with_bias:
                bias_sb = wpool.tile([1, 772], BF16, name="bias_sb")
                nc.scalar.dma_start(bias_sb[:], biasr_d.ap()[:, :])

            def xtv(kt, lo, hi):
                if lo < 512:
                    return xt_sb[kt, 0][:, lo:hi]
                if lo < 1024:
                    return xt_sb[kt, 1][:, lo - 512:hi - 512]
                return xt_sb[kt, 3][:, lo - 1024:hi - 1024]

            def wqkv(kt, hp, which):
                o = kt * 512 + hp * 256 + which * 128
                return wqk_sb[:, o:o + 128]

            def wvv(kt, hp):
                o = kt * 260 + hp * 130
                return wv_sb[:, o:o + 130]

            def auxv(hp, which):   # 0 q, 1 k, 2 v(130)
                o = hp * 640 + which * 128
                return aux_sb[0:1, o:o + (130 if which == 2 else 128)]

            def biasv(hp, which):
                o = hp * 386 + which * 128
                return bias_sb[0:1, o:o + (130 if which == 2 else 128)]

            # ---- per head-pair state ----
            qT = {}; kTt = {}; v_nat = {}; attnT = {}
            stats = {}; mrow = {}; drow = {}
            for hp in range(2):
                qT[hp] = bigp.tile([128, N], BF16, name=f"qT{hp}", tag="qT")
                kTt[hp] = bigp.tile([128, N], BF16, name=f"kT{hp}", tag="kT")
                v_nat[hp] = bigp.tile([128, NT * 130], BF16, name=f"vn{hp}",
                                      tag="vn")
                attnT[hp] = bigp.tile([128, N], BF16, name=f"aT{hp}",
                                      tag="aT")
                # ones cols for the PV denominators
                vv = v_nat[hp].rearrange("p (n c) -> p n c", c=65)
                nc.vector.memset(vv[:, :, 64:65], 1.0)

            # ========= phase A (projections + shared LN stats) =========
            def emit_gram(c):
                """token-gram diag blocks in fp8 DoubleRow: 0.5 cyc/row and
                256-deep contraction per matmul (4 pair-matmuls cover D)."""
                g_ps = psM.tile([128, 512], F32, name=f"g_{c}", tag="m")
                if c == 1:
                    # tokens 512:1023 are already on chip (wave 1): bf16
                    # gram lets the fp8 copy trail the whole x stream
                    for i in range(4):
                        t0 = c * 512 + i * 128
                        for g in range(2):
                            for kt in range(KT):
                                nc.tensor.matmul(
                                    g_ps[:, (i * 2 + g) * 64:
                                         (i * 2 + g + 1) * 64],
                                    xtv(kt, t0, t0 + 128),
                                    xtv(kt, t0 + g * 64, t0 + g * 64 + 64),
                                    start=(i == 0 and g == 0 and kt == 0),
                                    stop=(i == 3 and g == 1
                                          and kt == KT - 1),
                                    skip_group_check=True)
                    return g_ps
                xf = xf8_sb[0]
                # pair-major: DMA split s carries exactly k-tile pair s, so
                # the first accumulation pass starts on the first arrival
                for pr in range(KT // 2):
                    for i in range(4):
                        t0 = c * 512 + i * 128
                        for g in range(2):
                            nc.tensor.matmul(
                                g_ps[:, (i * 2 + g) * 64:(i * 2 + g + 1) * 64],
                                xf[:, 2 * pr:2 * pr + 2, t0:t0 + 128],
                                xf[:, 2 * pr:2 * pr + 2,
                                   t0 + g * 64:t0 + g * 64 + 64],
                                start=(pr == 0 and i == 0 and g == 0),
                                stop=(pr == KT // 2 - 1 and i == 3 and g == 1),
                                perf_mode=DR,
                                skip_group_check=True)
                return g_ps

            def emit_vproj(hp, c, half):
                """2 token tiles (half=0: tiles 0,1; half=1: tiles 2,3);
                per-tile cols: [v_h0 64 | v_h1 64 | mean | pad] = 130"""
                v_ps = psM.tile([128, 260], F32, name=f"v{hp}_{c}_{half}",
                                tag="m")
                for li in range(2):
                    i = half * 2 + li
                    t0 = c * 512 + i * 128
                    for kt in range(KT):
                        nc.tensor.matmul(
                            v_ps[:, li * 130:li * 130 + 130],
                            xtv(kt, t0, t0 + 128),
                            wvv(kt, hp),
                            start=(li == 0 and kt == 0), stop=False,
                            skip_group_check=True)
                return v_ps

            def emit_diag(c, g_ps):
                # stats cols: 0:4 mean, 4:8 rstd, 8:12 var, 12:16 std
                st = smallp.tile([128, 16], F32, name=f"st_{c}",
                                 tag="stats", bufs=4)
                stats[c] = st
                scr = smallp.tile([64, 64], F32, name=f"scr_{c}",
                                  tag="scr", bufs=2)
                for i in range(4):
                    for g in range(2):
                        nc.vector.scalar_tensor_tensor(
                            out=scr[:],
                            in0=g_ps[g * 64:(g + 1) * 64,
                                     (i * 2 + g) * 64:(i * 2 + g + 1) * 64],
                            scalar=1.0 / D,
                            in1=identf_sb[0:64, 0:64],
                            op0=ALU.mult, op1=ALU.mult,
                            accum_out=st[g * 64:(g + 1) * 64, 8 + i:9 + i])

            def emit_meanvar(c, v_a, v_b):
                st = stats[c]
                for half, v_ps in ((0, v_a), (1, v_b)):
                    vv = v_ps.rearrange("p (n c) -> p n c", c=130)
                    nc.vector.tensor_copy(
                        st[:, 2 * half:2 * half + 2]
                        .rearrange("p (n c) -> p n c", c=1),
                        vv[:, :, 128:129])
                sq = smallp.tile([128, 4], F32, name=f"sq_{c}", tag="sq",
                                 bufs=2)
                nc.vector.tensor_mul(sq[:], st[:, 0:4], st[:, 0:4])
                nc.vector.scalar_tensor_tensor(
                    out=st[:, 8:12], in0=st[:, 8:12], scalar=EPS, in1=sq[:],
                    op0=ALU.add, op1=ALU.subtract)
                # rstd = rsqrt(var) by Newton iteration on GPSIMD (mult/add
                # only).  LN input is unit-normal so var+eps is within
                # [0.7, 1.4]; three steps from y0=1 give ~1e-7 accuracy and
                # keep both ACT (exp-bound) and DVE off this chain.
                y = st[:, 4:8]
                t = smallp.tile([128, 4], F32, name=f"nw_{c}", tag="nw",
                                bufs=2)
                nc.gpsimd.tensor_scalar(out=y, in0=st[:, 8:12],
                                        scalar1=-0.5, scalar2=1.5,
                                        op0=ALU.mult, op1=ALU.add)
                for _ in range(2):
                    nc.gpsimd.tensor_mul(t[:], y, y)
                    nc.gpsimd.tensor_mul(t[:], t[:], st[:, 8:12])
                    nc.gpsimd.tensor_scalar(out=t[:], in0=t[:],
                                            scalar1=-0.5, scalar2=1.5,
                                            op0=ALU.mult, op1=ALU.add)
                    nc.gpsimd.tensor_mul(y, y, t[:])
                if with_bias:
                    # std = var * rstd
                    nc.gpsimd.tensor_mul(st[:, 12:16], st[:, 8:12], y)

            def emit_stsb_head(c):
                """mean row [1, 512] at partition 0 (matmul operands must
                sit at base partition 0): bf16 per-column transposes"""
                st = stats[c]
                if not with_bias:
                    stb = smallp.tile([128, 4], BF16, name=f"stb_{c}",
                                      tag="stb", bufs=2)
                    nc.vector.tensor_copy(stb[:], st[:, 0:4])
                    u_ps = psM.tile([128, 512], F32, name=f"u_{c}", tag="m")
                    ub = u_ps.bitcast(BF16)
                    for i in range(4):
                        nc.tensor.transpose(ub[0:1, i * 128:(i + 1) * 128],
                                            stb[:, i:i + 1], identb_sb)
                    row = smallp.tile([1, 512], BF16, name=f"row_{c}",
                                      tag="mrow", bufs=4)
                    nc.vector.tensor_copy(row[0:1, :], ub[0:1, 0:512])
                    mrow[c] = row[0:1, 0:512]
                    return
                u_ps = psM.tile([128, 512], F32, name=f"u_{c}", tag="m")
                for i in range(4):
                    nc.tensor.transpose(u_ps[0:1, i * 128:(i + 1) * 128],
                                        st[:, i:i + 1], identf_sb)
                row = smallp.tile([1, 512], BF16, name=f"row_{c}",
                                  tag="mrow", bufs=4)
                nc.vector.tensor_copy(row[0:1, :], u_ps[0:1, 0:512])
                mrow[c] = row[0:1, 0:512]
                if with_bias:
                    # transpose outputs must land on partition 0 (HW rule)
                    u2 = psM.tile([128, 512], F32, name=f"u2_{c}", tag="m")
                    for i in range(4):
                        nc.tensor.transpose(
                            u2[0:1, i * 128:(i + 1) * 128],
                            st[:, 12 + i:13 + i], identf_sb)
                    dr = smallp.tile([1, 512], BF16, name=f"dr_{c}",
                                     tag="drow", bufs=4)
                    nc.vector.tensor_copy(dr[0:1, :], u2[0:1, 0:512])
                    drow[c] = dr

            def emit_vtail(hp, c, v_a, v_b):
                """v rank1 (needs mean rows) + evict with per-partition rstd"""
                st = stats[c]
                for half, v_ps in ((0, v_a), (1, v_b)):
                    for li in range(2):
                        i = half * 2 + li
                        last = (li == 1)
                        nc.tensor.matmul(v_ps[:, li * 130:li * 130 + 130],
                                         mrow[c][:, i * 128:(i + 1) * 128],
                                         auxv(hp, 2),
                                         start=False,
                                         stop=last and not with_bias,
                                         skip_group_check=True)
                        if with_bias:
                            nc.tensor.matmul(v_ps[:, li * 130:li * 130 + 130],
                                             drow[c][0:1,
                                                     i * 128:(i + 1) * 128],
                                             biasv(hp, 2),
                                             start=False, stop=last,
                                             skip_group_check=True)
                    for li in range(2):
                        i = half * 2 + li
                        jb = (c * 4 + i) * 130
                        dst = v_nat[hp][:, jb:jb + 130].rearrange(
                            "p (h c) -> p h c", c=65)[:, :, 0:64]
                        nc.vector.tensor_scalar(
                            out=dst,
                            in0=v_ps[:, li * 130:li * 130 + 128].rearrange(
                                "p (h c) -> p h c", c=64),
                            scalar1=st[:, 4 + i:5 + i], scalar2=None,
                            op0=ALU.mult)

            qk_pr = {}; qk_qn = {}

            def emit_c0_ktmajor():
                """chunk 0 of pass 0 with all accumulations advancing
                k-tile-major, so PE work tracks the xt arrival order; the
                gram runs in bf16 off xt (xf8 lands after the first half)"""
                g_ps = psM.tile([128, 512], F32, name="g_0", tag="m")
                va = psM.tile([128, 260], F32, name="v0_0_0", tag="m")
                vb = psM.tile([128, 260], F32, name="v0_0_1", tag="m")
                prq = psS.tile([128, 512], F32, name="p00_0", tag="S")
                prk = psS.tile([128, 512], F32, name="p10_0", tag="S")
                for kt in range(KT):
                    for i in range(4):
                        t0 = i * 128
                        for g2 in range(2):
                            nc.tensor.matmul(
                                g_ps[:, (i * 2 + g2) * 64:
                                     (i * 2 + g2 + 1) * 64],
                                xtv(kt, t0, t0 + 128),
                                xtv(kt, t0 + g2 * 64, t0 + g2 * 64 + 64),
                                start=(kt == 0 and i == 0 and g2 == 0),
                                stop=(kt == KT - 1 and i == 3 and g2 == 1),
                                skip_group_check=True)
                    for half, v_ps in ((0, va), (1, vb)):
                        for li in range(2):
                            i = half * 2 + li
                            nc.tensor.matmul(
                                v_ps[:, li * 130:li * 130 + 130],
                                xtv(kt, i * 128, (i + 1) * 128),
                                wvv(kt, 0),
                                start=(kt == 0 and li == 0),
                                stop=False, skip_group_check=True)
                    for i in range(4):
                        nc.tensor.matmul(
                            prq[:, i * 128:(i + 1) * 128],
                            xtv(kt, i * 128, (i + 1) * 128),
                            wqkv(kt, 0, 0),
                            start=(kt == 0 and i == 0), stop=False,
                            skip_group_check=True)
                    if with_bias:
                        for i in range(4):
                            nc.tensor.matmul(
                                prk[:, i * 128:(i + 1) * 128],
                                xtv(kt, i * 128, (i + 1) * 128),
                                wqkv(kt, 0, 1),
                                start=(kt == 0 and i == 0), stop=False,
                                skip_group_check=True)
                    else:
                        nc.tensor.matmul(
                            prk[:], wqkv(kt, 0, 1), xtv(kt, 0, 512),
                            start=(kt == 0), stop=False,
                            skip_group_check=True)
                qk_pr[0, 0, 0] = prq
                qk_pr[0, 0, 1] = prk
                return g_ps, va, vb

            def emit_qk_mm(hp, c, which, half):
                """natural-layout projection matmuls (psS ring; no stats
                deps). half 0 = token tiles 0-1, half 1 = tiles 2-3"""
                if half == 0:
                    pr = psS.tile([128, 512], F32, name=f"p{which}{hp}_{c}",
                                  tag="S")
                    qk_pr[hp, c, which] = pr
                else:
                    pr = qk_pr[hp, c, which]
                for i in (0, 1) if half == 0 else (2, 3):
                    t0 = c * 512 + i * 128
                    for kt in range(KT):
                        nc.tensor.matmul(
                            pr[:, i * 128:(i + 1) * 128],
                            xtv(kt, t0, t0 + 128),
                            wqkv(kt, hp, which),
                            start=(i == 0 and kt == 0), stop=False,
                            skip_group_check=True)

            def emit_qk_fin(hp, c, which):
                """rank-1 LN mean correction + per-partition rstd evict"""
                st = stats[c]
                pr = qk_pr[hp, c, which]
                for i in range(4):
                    last = (i == 3)
                    nc.tensor.matmul(pr[:, i * 128:(i + 1) * 128],
                                     mrow[c][:, i * 128:(i + 1) * 128],
                                     auxv(hp, which),
                                     start=False,
                                     stop=last and not with_bias,
                                     skip_group_check=True)
                    if with_bias:
                        nc.tensor.matmul(pr[:, i * 128:(i + 1) * 128],
                                         drow[c][0:1, i * 128:(i + 1) * 128],
                                         biasv(hp, which),
                                         start=False, stop=last,
                                         skip_group_check=True)
                qn = smallp.tile([128, 512], BF16, name=f"qn{which}{hp}_{c}",
                                 tag=f"qn{which}", bufs=2)
                for i in range(4):
                    # 2 on ACT: the q transpose waits on these and the DVE
                    # queue is the longer one in phase-A windows
                    if i < 2:
                        nc.scalar.mul(qn[:, i * 128:(i + 1) * 128],
                                      pr[:, i * 128:(i + 1) * 128],
                                      st[:, 4 + i:5 + i])
                    else:
                        nc.vector.tensor_scalar(
                            out=qn[:, i * 128:(i + 1) * 128],
                            in0=pr[:, i * 128:(i + 1) * 128],
                            scalar1=st[:, 4 + i:5 + i], scalar2=None,
                            op0=ALU.mult)
                qk_qn[hp, c, which] = qn

            def emit_qk_tr(hp, c, which):
                """transpose natural [t, hd] tiles into qT/kT"""
                qn = qk_qn[hp, c, which]
                tr = psM.tile([128, 512], BF16, name=f"tr{which}{hp}_{c}",
                              tag="m")
                for i in range(4):
                    nc.tensor.transpose(tr[:, i * 128:(i + 1) * 128],
                                        qn[:, i * 128:(i + 1) * 128],
                                        identb_sb)
                dst = qT[hp] if which == 0 else kTt[hp]
                nc.vector.tensor_copy(dst[:, c * 512:(c + 1) * 512], tr[:])

            def emit_k_mm(hp, c, half):
                """k projected directly transposed: out [hd, t] with the
                weight tile stationary; no separate PE transpose, and the
                per-token rstd is deferred to the exp's per-partition scale"""
                if half == 0:
                    pr = psS.tile([128, 512], F32, name=f"pk{hp}_{c}",
                                  tag="S")
                    qk_pr[hp, c, 1] = pr
                else:
                    pr = qk_pr[hp, c, 1]
                t0 = c * 512
                for kt in range(KT // 2 * half, KT // 2 * (half + 1)):
                    nc.tensor.matmul(
                        pr[:], wqkv(kt, hp, 1), xtv(kt, t0, t0 + 512),
                        start=(kt == 0), stop=False, skip_group_check=True)

            def emit_k_fin(hp, c):
                """rank-1 LN mean correction (outer product via K=1 matmul)
                + plain bf16 eviction into kT"""
                pr = qk_pr[hp, c, 1]
                for i in range(4):
                    nc.tensor.matmul(pr[:, i * 128:(i + 1) * 128],
                                     auxv(hp, 1),
                                     mrow[c][:, i * 128:(i + 1) * 128],
                                     start=False, stop=(i == 3),
                                     skip_group_check=True)
                nc.vector.tensor_copy(kTt[hp][:, c * 512:(c + 1) * 512],
                                      pr[:])

            def gen_A(hp):
                """generator emitting phase A in fine-grained steps; yields
                the PE-cycle estimate of what it just emitted (drive uses it
                to budget filler insertion), or ("ready", hp, c) markers.
                Pass 0 also produces the shared LN statistics."""
                if hp == 0:
                    g, va, vb = emit_c0_ktmajor()
                    emit_diag(0, g)
                    yield 2080
                else:
                    va = emit_vproj(1, 0, 0)
                    yield 2080
                    vb = emit_vproj(1, 0, 1)
                    yield 2080
                for c in range(NCH):
                    if hp == 0:
                        emit_meanvar(c, va, vb)
                        yield 0
                        if STAGE < 3:
                            if c + 1 < NCH:
                                g = emit_gram(c + 1)
                                emit_diag(c + 1, g)
                                va = emit_vproj(0, c + 1, 0)
                                vb = emit_vproj(0, c + 1, 1)
                            continue
                        emit_stsb_head(c)
                        yield 256
                    if not (hp == 0 and c == 0):
                        emit_qk_mm(hp, c, 0, 0)
                        yield 2048
                        emit_qk_mm(hp, c, 0, 1)
                    yield 2048
                    emit_vtail(hp, c, va, vb)
                    yield 520
                    emit_qk_fin(hp, c, 0)
                    yield 1024
                    if with_bias:
                        if not (hp == 0 and c == 0):
                            emit_qk_mm(hp, c, 1, 0)
                            yield 2048
                            emit_qk_mm(hp, c, 1, 1)
                            yield 2048
                        emit_qk_fin(hp, c, 1)
                        yield 1024
                        emit_qk_tr(hp, c, 0)
                        yield 512
                        emit_qk_tr(hp, c, 1)
                        yield ("ready", hp, c)
                    else:
                        if not (hp == 0 and c == 0):
                            emit_k_mm(hp, c, 0)
                            yield 2048
                            emit_k_mm(hp, c, 1)
                            yield 2048
                        emit_k_fin(hp, c)
                        yield 1024
                        emit_qk_tr(hp, c, 0)
                        yield ("ready", hp, c)
                    if c + 1 < NCH:
                        if hp == 0:
                            g = emit_gram(c + 1)
                            emit_diag(c + 1, g)
                            yield 1024
                        va = emit_vproj(hp, c + 1, 0)
                        yield 2080
                        vb = emit_vproj(hp, c + 1, 1)
                        yield 2080

            # =============== phase B (attention) ===============
            def jt_off(c4, jt):
                return 0 if jt < 4 * c4 else (jt - 4 * c4) * 128

            def emit_sblk(hp, c4, jt, h):
                """S block for one (jt, head); exp; diag mask."""
                o = jt_off(c4, jt)
                w = 512 - o
                sp = psS.tile([128, 512], F32, name=f"s{hp}{c4}{jt}{h}",
                              tag="S")
                nc.tensor.matmul(
                    sp[:, 0:w],
                    kTt[hp][h * 64:(h + 1) * 64, jt * 128:(jt + 1) * 128],
                    qT[hp][h * 64:(h + 1) * 64, c4 * 512 + o:(c4 + 1) * 512],
                    start=True, stop=True)
                p = ppool.tile([128, 512], BF16, name=f"e{hp}{c4}{jt}{h}",
                               tag="p")
                # kT is evicted unscaled; its per-key rstd rides the exp's
                # per-partition scale (keys sit on partitions here)
                sc = (1.0 if with_bias
                      else stats[jt // 4][:, 4 + jt % 4:5 + jt % 4])
                nc.scalar.activation(p[:, 0:w], sp[:, 0:w], AF.Exp, scale=sc)
                if jt >= 4 * c4:   # diagonal block: mask first 128 cols
                    # p is SBUF so GPSIMD may touch it; keeps DVE clear
                    nc.gpsimd.tensor_mul(p[:, 0:128], p[:, 0:128], tri_sb[:])
                return p, o

            def emit_pv(hp, c4, at_ps, p, jt, o, h):
                ils = [il for il in range(4) if 4 * c4 + il >= jt]
                if jt >= 4 * c4 and jt != 0 and len(ils) > 1:
                    # masked tile last (jt==0 keeps order: its il0/il2 writes
                    # carry the start flags that mark the psum banks)
                    ils = ils[1:] + ils[:1]
                for il in ils:
                    it = 4 * c4 + il
                    lo = il * 128 - o
                    abase = (il % 2) * 130 + (il // 2) * 512 + h * 65
                    nc.tensor.matmul(
                        at_ps[:, abase:abase + 65],
                        p[:, lo:lo + 128],
                        v_nat[hp][:, jt * 130 + h * 65:
                                  jt * 130 + h * 65 + 65],
                        start=(jt == 0 and h == 0 and il % 2 == 0),
                        stop=(jt == it),
                        skip_group_check=True)

            attn_an = {}

            def _at_views(at_ps):
                """strided views of at_ps [128, 1024] as [p, a, b, h, c]
                with col = 512a + 130b + 65h + c  (il = 2a + b)"""
                v = at_ps.rearrange("p (a r) -> p a r", a=2)[:, :, 0:260] \
                    .rearrange("p a (b r) -> p a b r", b=2) \
                    .rearrange("p a b (h c) -> p a b h c", h=2)
                return v[:, :, :, :, 0:64], v[:, :, :, :, 64:65]

            def emit_norm_il(hp, c4, at_ps, il):
                """softmax normalize one query tile (DVE)"""
                abase = (il % 2) * 130 + (il // 2) * 512
                rcp = smallp.tile([128, 2], F32, name=f"rc{hp}{c4}{il}",
                                  tag="rcp", bufs=4)
                nc.vector.reciprocal(rcp[:, 0:1],
                                     at_ps[:, abase + 64:abase + 65])
                nc.vector.reciprocal(rcp[:, 1:2],
                                     at_ps[:, abase + 129:abase + 130])
                an = smallp.tile([128, 128], BF16, name=f"an{hp}{c4}{il}",
                                 tag="an", bufs=4)
                for h in range(2):
                    nc.vector.tensor_scalar(
                        out=an[:, h * 64:(h + 1) * 64],
                        in0=at_ps[:, abase + h * 65:abase + h * 65 + 64],
                        scalar1=rcp[:, h:h + 1], scalar2=None,
                        op0=ALU.mult)
                attn_an[hp, c4, il] = an

            def emit_attn_norm(hp, c4, at_ps):
                """fused normalize: ONE reciprocal over the 8 denominator
                columns + ONE broadcast multiply over all 512 attention
                columns (vs 8 + 8 per-il DVE instructions)"""
                pv, dv = _at_views(at_ps)
                rcp = smallp.tile([128, 8], F32, name=f"rcf{hp}{c4}",
                                  tag="rcpf", bufs=2)
                rcv = rcp.rearrange("p (a b h c) -> p a b h c", a=2, b=2, h=2)
                nc.vector.reciprocal(rcv, dv)
                an = smallp.tile([128, 512], BF16, name=f"anf{hp}{c4}",
                                 tag="anf", bufs=2)
                anv = an.rearrange("p (a b h c) -> p a b h c", a=2, b=2, h=2)
                nc.vector.tensor_tensor(
                    anv, pv, rcv.broadcast_to([128, 2, 2, 2, 64]), ALU.mult)
                for il in range(4):
                    attn_an[hp, c4, il] = an[:, il * 128:(il + 1) * 128]

            def emit_attn_tr(hp, c4):
                """transpose normalized tiles into attnT"""
                tr_ps = psM.tile([128, 512], BF16, name=f"tr{hp}{c4}",
                                 tag="m")
                for il in range(4):
                    nc.tensor.transpose(tr_ps[:, il * 128:(il + 1) * 128],
                                        attn_an[hp, c4, il][:], identb_sb)
                nc.vector.tensor_copy(attnT[hp][:, c4 * 512:(c4 + 1) * 512],
                                      tr_ps[:])

            def emit_outproj(it, eng_pick, on_act=False):
                """y tile: BOTH head-pairs' contributions accumulate in the
                same PSUM before one eviction + one DMA (emitted only once
                pass 1's attnT for this tile exists)"""
                y_sb = smallp.tile([128, D], BF16, name=f"ys_{it}",
                                   tag="ysb", bufs=3)
                for e in range(2):
                    y_ps = psM.tile([128, 512], F32, name=f"y{it}_{e}",
                                    tag="m")
                    for hp in range(2):
                        nc.tensor.matmul(
                            y_ps[:],
                            attnT[hp][:, it * 128:(it + 1) * 128],
                            wo_sb[:, hp * D + e * 512:hp * D + (e + 1) * 512],
                            start=(hp == 0), stop=(hp == 1),
                            skip_group_check=True)
                    # GPSIMD cannot read PSUM; evictions live on DVE so ACT
                    # stays exp-only -- except the drain tail, where the
                    # exps are done and DVE is the backlogged queue
                    if on_act:
                        nc.scalar.copy(y_sb[:, e * 512:(e + 1) * 512],
                                       y_ps[:])
                    else:
                        nc.vector.tensor_copy(
                            y_sb[:, e * 512:(e + 1) * 512], y_ps[:])
                    # per-half DMA: the first half ships while the second
                    # half is still evicting (shortens the drain tail)
                    nc.sync.dma_start(
                        y_d.ap()[it * 128:(it + 1) * 128,
                                 e * 512:(e + 1) * 512],
                        y_sb[:, e * 512:(e + 1) * 512])

            def gen_B(hp, deferred, late):
                for c4 in range(NCH):
                    yield ("need", hp, c4)
                    njt = 4 * c4 + 4
                    at_ps = psA.tile([128, 1024], F32, name=f"at{hp}{c4}",
                                     tag="attn")
                    last_unit = (STAGE >= 5 and hp == 1 and c4 == NCH - 1)
                    tr_last = [None]

                    def stream_il(il, hp=hp, c4=c4, at_ps=at_ps,
                                  tr_last=tr_last):
                        """last chunk streams per-tile finish + outproj so
                        the tail drains early"""
                        if tr_last[0] is None:
                            tr_last[0] = psM.tile([128, 512], BF16,
                                                  name=f"trL{hp}{c4}",
                                                  tag="m")
                        emit_norm_il(hp, c4, at_ps, il)
                        nc.tensor.transpose(
                            tr_last[0][:, il * 128:(il + 1) * 128],
                            attn_an[hp, c4, il][:], identb_sb)
                        it = 4 * c4 + il
                        nc.vector.tensor_copy(
                            attnT[hp][:, it * 128:(it + 1) * 128],
                            tr_last[0][:, il * 128:(il + 1) * 128])
                        if STAGE >= 6:
                            emit_outproj(it, il, on_act=True)

                    # PV lags its S by one jt for h0 and two jts for h1:
                    # the h1 exp lands ~1.4us after its S pair, which is
                    # more than one jt of PE work, so a 1-jt lag alone
                    # stalls the in-order PE queue on the h1 PVs.
                    pend = []   # (p, jt, o, h) awaiting PV emission
                    for jt in range(njt):
                        cur = []
                        for h in range(2):
                            p, o = emit_sblk(hp, c4, jt, h)
                            cur.append((p, jt, o, h))
                        # ask drive to insert ~FILL_CYC PE-cycles of filler
                        # (deferred + phase-A steps) so the PV matmuls below
                        # don't reach the head of the in-order PE queue
                        # before their exp has landed
                        yield ("fill", FILL_CYC)
                        if STAGE >= 5:
                            emitted_il = None
                            for (p, j, o, h) in list(pend):
                                if j == jt - 1:
                                    continue   # both heads lag two jts
                                emit_pv(hp, c4, at_ps, p, j, o, h)
                                pend.remove((p, j, o, h))
                                if h == 1 and last_unit and j >= 4 * c4:
                                    emitted_il = j - 4 * c4
                            if emitted_il is not None:
                                stream_il(emitted_il)
                        pend.extend(cur)
                    if STAGE >= 5:
                        for (p, j, o, h) in pend:
                            emit_pv(hp, c4, at_ps, p, j, o, h)
                            if (h == 1 and last_unit and j >= 4 * c4
                                    and j - 4 * c4 < 3):
                                stream_il(j - 4 * c4)
                        if last_unit:
                            stream_il(3)
                        else:
                            deferred.append((0,
                                lambda hp=hp, c4=c4, at=at_ps:
                                emit_attn_norm(hp, c4, at)))
                            deferred.append((512,
                                lambda hp=hp, c4=c4: emit_attn_tr(hp, c4)))
                            if STAGE >= 6 and hp == 1:
                                # y needs both passes' attnT: emit the first
                                # three chunks' worth during pass-1 c4s
                                for il in range(4):
                                    deferred.append((2048,
                                        lambda it=4 * c4 + il, il=il:
                                        emit_outproj(it, il)))
                                if c4 == NCH - 2:
                                    # pass-0 tiles beyond pass-1's c4 range
                                    # never get a c4 of their own; none
                                    # exist (ranges match), so nothing here
                                    pass

            # =============== master schedule ===============
            def chain(*gens):
                for g in gens:
                    yield from g

            def drive(bgen, agen, deferred):
                """interleave B with budgeted filler: on each B ("fill", n)
                request, insert deferred items and phase-A steps totalling
                ~n PE-cycles. Never let B emit reads of phase-A tiles before
                their writers exist: B announces ("need", hp, c4); A
                announces ("ready", hp, c)."""
                ready = set()
                a_done = [False]

                def pump_a():
                    """advance A one step; returns its PE-cycle estimate"""
                    if a_done[0]:
                        return 0
                    try:
                        item = next(agen)
                    except StopIteration:
                        a_done[0] = True
                        return 0
                    if isinstance(item, tuple):
                        ready.add(item[1:])
                        return 0
                    return item or 0

                while True:
                    try:
                        item = next(bgen)
                    except StopIteration:
                        break
                    if item is not None and item[0] == "need":
                        while item[1:] not in ready and not a_done[0]:
                            pump_a()
                            if deferred:
                                deferred.pop(0)[1]()
                        assert item[1:] in ready, f"A never produced {item}"
                    else:
                        need = item[1] if item else 0
                        while need > 0:
                            if deferred:
                                cyc, fn = deferred.pop(0)
                                fn()
                                need -= max(cyc, 64)
                            if need > 0 and not a_done[0]:
                                need -= max(pump_a(), 64)
                            elif not deferred:
                                break
                while not a_done[0]:
                    pump_a()

            deferred = []
            late = []
            if STAGE >= 4:
                aq = chain(gen_A(0), gen_A(1))
                bq = chain(gen_B(0, deferred, late),
                           gen_B(1, deferred, late))
                drive(bq, aq, deferred)
                while deferred:
                    deferred.pop(0)[1]()
                while late:
                    late.pop(0)[1]()
            else:
                for _ in chain(gen_A(0), gen_A(1)):
                    pass

    nc.compile()
    return nc


_PROG_CACHE = {}


def _get_program(with_bias):
    key = (with_bias, STAGE)
    if key not in _PROG_CACHE:
        _PROG_CACHE[key] = _build_program(with_bias)
    return _PROG_CACHE[key]


def kernel(x, ln_g, ln_b, lnc_g, lnc_b, Wq, Wkv, Wo):
    global LAST_RESULTS
    x = np.ascontiguousarray(np.asarray(x, dtype=np.float32))
    ln_g = np.asarray(ln_g, np.float32); ln_b = np.asarray(ln_b, np.float32)
    lnc_g = np.asarray(lnc_g, np.float32); lnc_b = np.asarray(lnc_b, np.float32)
    Wq = np.asarray(Wq, np.float32); Wkv = np.asarray(Wkv, np.float32)
    Wo = np.asarray(Wo, np.float32)
    scale = DH ** -0.5

    with_bias = bool(np.any(ln_b) or np.any(lnc_b))
    nc = _get_program(with_bias)

    xt_f = np.ascontiguousarray(np.transpose(x, (0, 2, 1)))   # [B, D, N]
    xt = xt_f.astype(NPBF)
    # [b, p, kt*N + t] = x^T[b, kt*128+p, t], fp8 e4m3 for the DoubleRow gram
    xf8 = np.ascontiguousarray(
        xt_f.reshape(B, KT, 128, N).transpose(0, 2, 1, 3)
        .reshape(B, 128, KT * N)).astype(ml_dtypes.float8_e4m3)
    tri = np.triu(np.ones((128, 128), np.float32)).astype(NPBF)
    identb = np.eye(128, dtype=np.float32).astype(NPBF)
    identf = np.eye(128, dtype=np.float32)

    in_maps = []
    for c in range(NCORES):
        b = c // (NCORES // B)            # batch of this core
        g = c % (NCORES // B)             # head-group (4 heads)
        cs = slice(g * HG * DH, (g + 1) * HG * DH)
        Wq_eff = ln_g[:, None] * Wq[:, cs] * scale          # [D, 256]
        Wk_eff = lnc_g[:, None] * Wkv[:, :H * DH][:, cs]
        Wv_eff = lnc_g[:, None] * Wkv[:, H * DH:][:, cs]
        # per k-tile: [q_hp0 | k_hp0 | q_hp1 | k_hp1] (128 each)
        wqk = np.concatenate([Wq_eff[:, 0:128], Wk_eff[:, 0:128],
                              Wq_eff[:, 128:256], Wk_eff[:, 128:256]], axis=1)
        wqk = np.ascontiguousarray(
            wqk.reshape(KT, 128, 512).transpose(1, 0, 2).reshape(128, KT * 512))
        # per k-tile: [v_hp0 128 | 1/D | pad | v_hp1 128 | 1/D | pad]
        oneD = np.full((D, 1), 1.0 / D)
        zD = np.zeros((D, 1), np.float32)
        wv = np.concatenate([Wv_eff[:, 0:128], oneD, zD,
                             Wv_eff[:, 128:256], oneD, zD], axis=1)
        wv = np.ascontiguousarray(
            wv.reshape(KT, 128, 260).transpose(1, 0, 2).reshape(128, KT * 260))
        Wo_cs = Wo[cs, :]                                    # [256, D]
        wo = np.concatenate([Wo_cs[0:128, :], Wo_cs[128:256, :]], axis=1)
        aux = np.zeros((1, 1280), np.float32)
        for hp in range(2):
            hs = slice(hp * 128, (hp + 1) * 128)
            aux[0, hp * 640 + 0:hp * 640 + 128] = -Wq_eff[:, hs].sum(0)
            aux[0, hp * 640 + 128:hp * 640 + 256] = -Wk_eff[:, hs].sum(0)
            aux[0, hp * 640 + 256:hp * 640 + 384] = -Wv_eff[:, hs].sum(0)
            aux[0, hp * 640 + 512:hp * 640 + 640] = 1.0
        m = {
            "xt": xt[b],
            "xf8": xf8[b],
            "wqk": wqk.astype(NPBF),
            "wv": wv.astype(NPBF),
            "wo": np.ascontiguousarray(wo).astype(NPBF),
            "aux": aux.astype(NPBF),
            "tri": tri, "identb": identb, "identf": identf,
        }
        if with_bias:
            br = np.zeros((1, 772), np.float32)
            for hp in range(2):
                hs = slice(g * HG * DH + hp * 128, g * HG * DH + (hp + 1) * 128)
                br[0, hp * 386 + 0:hp * 386 + 128] = ln_b @ Wq[:, hs] * scale
                br[0, hp * 386 + 128:hp * 386 + 256] = \
                    lnc_b @ Wkv[:, :H * DH][:, hs]
                br[0, hp * 386 + 256:hp * 386 + 384] = \
                    lnc_b @ Wkv[:, H * DH:][:, hs]
            m["biasr"] = br.astype(NPBF)
        in_maps.append(m)

    res = run_bass_kernel_spmd(nc, in_maps, core_ids=list(range(NCORES)),
                               trace=TRACE, **TRACE_KWARGS)
    LAST_RESULTS = res
    ys = []
    gpb = NCORES // B
    for b in range(B):
        yb = res.results[b * gpb]["y"].astype(np.float32)
        for g in range(1, gpb):
            yb += res.results[b * gpb + g]["y"].astype(np.float32)
        ys.append(yb)
    return np.stack(ys)
